# revision 59
# baseline (speedup 1.0000x reference)
"""Trainium2 Bass kernel for multi-head attention (nn_Attention_54984171323822).

Reference computation (fp32):
    qkv = x @ w_qkv.T + b_qkv            # [B, N, 3*1024]
    q, k, v -> 16 heads x 64
    attn = softmax(q k^T / 8) v          # per head
    out = attn_flat @ w_out.T + b_out    # [B, N, 1024]

Shapes: B=4, N=2048, HIDDEN=1024, 16 heads x 64.

Sharding (8 NeuronCores): DP=4 over batch x TP=2 over heads. Core c handles
batch c//2 and heads (c%2)*8..(c%2)*8+8. No device collectives: each core
emits a partial output-projection [2048, 1024]; the host sums the TP pairs
and adds b_out (linear, so it commutes).

Per-core device program — one fused software pipeline:
  * Attention runs per (head, q-half, kc) "unit": two K=128 scores matmuls
    against zero-padded K bands (kz_e/kz_o; the sibling head's rows
    multiply zeros, keeping every attention matmul in the same (128,128)
    PE tile mode), one ScalarE exp([128,1024]) reading the two-bank score
    tile, and PV matmuls lagged 2-3 units behind (V carries a ones column
    -> softmax denominator in psum row 64).  ScalarE is the bottleneck
    (33.5M exps/core, 1 elem/lane/cycle, exp exists only there) and runs
    essentially back-to-back through the attention window.
  * QKV projection runs as packets interleaved into the attention stream
    (psum borrows score-ring slots): pair-0 k/q up front (moving any of
    them into the queue re-triggers a Tile-scheduler psum-group race!),
    V token-chunks just-in-time during head 0, pairs 1-3 spread across
    their deadline windows as M=256 half-packets.
  * Output projection at the tail (needs all heads); partials leave in
    bf16 to halve the output DMA.

The no-max-subtraction softmax is safe here: logits are ~N(0, 0.66^2) after
the 1/8 scale, so exp() stays well within fp32/bf16 range.
"""

import sys

sys.path.insert(0, "/opt/trn_rl_repo")

import numpy as np
import ml_dtypes

import concourse.bass as bass
import concourse.bacc as bacc
import concourse.tile as tile
from concourse import mybir
from concourse import bass_utils

N_CORES = 8
B = 4
N = 2048
HIDDEN = 1024
N_HEADS = 16
HEAD_DIM = 64
HPC = N_HEADS // 2          # heads per core (TP=2)
EC = HPC * HEAD_DIM         # 512 attention dims per core
TC = N // 128               # 16 token chunks
DC = HIDDEN // 128          # 8 hidden chunks
NPAIR = HPC // 2            # 4 head pairs per core
SCALE = HEAD_DIM ** -0.5

M1024 = False                # single M=1024 matmuls for scores/PV

BF16 = mybir.dt.bfloat16
F32 = mybir.dt.float32
NP_BF16 = ml_dtypes.bfloat16


def _build_kernel_body(nc, tc_ctx, ios, dbg=None):
    import contextlib

    xT, wqkvT, bias_qk, bias_v, w_outT, out = ios
    tc = tc_ctx
    ctx = contextlib.ExitStack()
    with ctx:
        const = ctx.enter_context(tc.tile_pool(name="const", bufs=1))
        work = ctx.enter_context(tc.tile_pool(name="work", bufs=3))
        etp = ctx.enter_context(tc.tile_pool(name="etp", bufs=10))
        small = ctx.enter_context(tc.tile_pool(name="small", bufs=2))
        stp = ctx.enter_context(tc.tile_pool(name="stp", bufs=2, space="PSUM"))
        pvp = ctx.enter_context(tc.tile_pool(name="pvp", bufs=2, space="PSUM"))

        # ---- resident SBUF tensors ----
        xT_src = xT.ap().rearrange("(c p) t -> c p t", p=128)
        wq_src = wqkvT.ap().rearrange("(c p) e -> c p e", p=128)
        xT_c = []
        wq_c = []
        for dc in range(DC):
            # w columns are pair-major: [q0|k0|q1|k1|q2|k2|q3|k3|v(512)].
            # 3 DMAs per chunk so pair-0 k/q and v can start without waiting
            # for pairs 1-3 weights.
            wt = const.tile([128, 3 * EC], BF16, name=f"wq{dc}", tag=f"wq{dc}")
            nc.scalar.dma_start(out=wt[:, 0:256], in_=wq_src[dc][:, 0:256])
            nc.gpsimd.dma_start(out=wt[:, 1024:1536], in_=wq_src[dc][:, 1024:1536])
            nc.scalar.dma_start(out=wt[:, 256:1024], in_=wq_src[dc][:, 256:1024])
            wq_c.append(wt)
            xt = const.tile([128, N], BF16, name=f"xc{dc}", tag=f"xc{dc}")
            nc.sync.dma_start(out=xt[:], in_=xT_src[dc])
            xT_c.append(xt)
        bqk_sb = const.tile([128, 8], F32, name="bqk_sb", tag="bqk_sb")
        nc.sync.dma_start(out=bqk_sb[:], in_=bias_qk.ap())
        # bias_v broadcast to all partitions ([1, 520] dram, partition step 0)
        bv_sb = const.tile([128, HPC * 65], BF16, name="bv_sb", tag="bv_sb")
        bv_ap = bias_v.ap()
        bv_bcast = bass.AP(tensor=bv_ap.tensor, offset=bv_ap.offset,
                           ap=[[0, 128], [1, HPC * 65]])
        nc.gpsimd.dma_start(out=bv_sb[:], in_=bv_bcast)
        wo_sb = const.tile([128, EC // 128, HIDDEN], BF16, name="wo_sb", tag="wo_sb")
        nc.sync.dma_start(out=wo_sb[:], in_=w_outT.ap().rearrange("(c p) e -> p c e", p=128))

        # qkT holds q chunks only (chunk j = q of pair j).  K lives in
        # zero-padded band tensors so scores run as K=128 matmuls in the
        # same (128,128) PE tile mode as PV/packets (no mode switches):
        # kz_e rows 0-63 = even head's k, rows 64-127 = 0; kz_o mirrored.
        qkT = const.tile([128, 4, N], BF16, name="qkT", tag="qkT")
        kz_e = const.tile([128, 4, N], BF16, name="kz_e", tag="kz_e")
        kz_o = const.tile([128, 4, N], BF16, name="kz_o", tag="kz_o")
        nc.vector.memset(kz_e[64:128, :, :], 0.0)
        nc.vector.memset(kz_o[0:64, :, :], 0.0)
        vpp = const.tile([128, TC, HPC * 65], BF16, name="vpp", tag="vpp")
        attnT_c = [const.tile([128, N], BF16, name=f"attnT{i}", tag=f"attnT{i}")
                   for i in range(EC // 128)]

        ones64 = const.tile([128, 64], F32, name="ones64", tag="ones64")
        nc.vector.memset(ones64[:], 1.0)
        # ones columns of V'' (col h*65+64 of every token chunk)
        ones_ap = vpp[:].rearrange("p t (h u) -> p t h u", u=65)[:, :, :, 64:65]
        nc.vector.memset(ones_ap, 1.0)

        # ------------------------------------------------------------------
        # QKV packet machinery.  A packet computes one (chunk, ti) of qkT
        # (plus bias add) or one ti of V''.  The psum accumulator borrows a
        # slot of the score-tile ring (stp).
        # ------------------------------------------------------------------
        def qk_packet(ec, ti, half=None):
            # chunk ec (pair-major q/k), token range of 512 (or 256) tokens
            wcol = (ec // 2) * 256 + (ec % 2) * 128
            t0 = ti * 512 + (0 if half in (None, 0) else 256)
            tn = 512 if half is None else 256
            ps = stp.tile([128, 1024], F32, name="st", tag="st")
            for dc in range(DC):
                for eh in range(2):
                    nc.tensor.matmul(
                        ps[eh * 64:(eh + 1) * 64, 0:tn],
                        wq_c[dc][:, wcol + eh * 64:wcol + (eh + 1) * 64],
                        xT_c[dc][:, t0:t0 + tn],
                        start=(dc == 0), stop=(dc == DC - 1),
                        skip_group_check=True,
                    )
            j = ec // 2
            ts = slice(t0, t0 + tn)
            if ec % 2 == 0:
                nc.vector.tensor_scalar_add(
                    qkT[:, j, ts], ps[:, 0:tn], bqk_sb[:, ec:ec + 1])
            else:
                nc.vector.tensor_scalar_add(
                    kz_e[0:64, j, ts], ps[0:64, 0:tn], bqk_sb[0:64, ec:ec + 1])
                nc.vector.tensor_scalar_add(
                    kz_o[64:128, j, ts], ps[64:128, 0:tn], bqk_sb[64:128, ec:ec + 1])

        def v_packet(ti):
            # V'' token chunk ti: [128 tok, 512 vdims (+ ones cols)]
            ps = stp.tile([128, 1024], F32, name="st", tag="st")
            for dc in range(DC):
                for th in range(2):
                    nc.tensor.matmul(
                        ps[th * 64:(th + 1) * 64, 0:512],
                        xT_c[dc][:, ti * 128 + th * 64:ti * 128 + (th + 1) * 64],
                        wq_c[dc][:, 2 * EC:3 * EC],
                        start=(dc == 0), stop=(dc == DC - 1),
                        skip_group_check=True,
                    )
            v_out = vpp[:, ti].rearrange("p (h u) -> p h u", u=65)[:, :, 0:64]
            v_in = ps[:, 0:512].rearrange("p (h u) -> p h u", u=64)
            v_bias = bv_sb[:].rearrange("p (h u) -> p h u", u=65)[:, :, 0:64]
            nc.vector.tensor_tensor(out=v_out, in0=v_in, in1=v_bias,
                                    op=mybir.AluOpType.add)

        # Packet queue.  Minimal prefix of pair 0 runs before the attention
        # loop; V'' chunks and remaining q/k interleave (V early: pair-0 PV
        # consumes them in kc order).
        packets = []
        for ti in range(TC):
            packets.append(lambda ti=ti: v_packet(ti))
        for j in range(1, NPAIR):
            for ec in (2 * j + 1, 2 * j):     # k first, then q
                for ti in range(4):
                    for hf in range(2):
                        packets.append(
                            lambda ec=ec, ti=ti, hf=hf: qk_packet(ec, ti, hf))
        pkt_i = 0

        def drain_packets(k):
            nonlocal pkt_i
            for _ in range(k):
                if pkt_i < len(packets):
                    packets[pkt_i]()
                    pkt_i += 1

        # ---- pair-0 k/q up front ----
        for ec in (1, 0):
            for ti in range(4):
                qk_packet(ec, ti)

        # ------------------------------------------------------------------
        # Attention: units of (head h, q-half qh, key chunk kc).
        # ------------------------------------------------------------------
        pending_pv = []   # (et, h, qh, kc, pv)

        def normalize(h, qh, pv):
            # attnT_h[d, q] = oT'[d, q] * recip(oT'[64, q])
            j = h // 2
            rec = small.tile([65, 2, 512], F32, name="rec", tag="rec", bufs=2)
            for qi in range(2):
                nc.vector.reciprocal_approx_fast(out=rec[0:65, qi, :],
                                                 in_=pv[:, qi, :])
            if h == HPC - 1 and qh == 1:
                # final tail: broadcast the recip row via a K=1 PE matmul
                # (st ring is idle by now) — faster than the DMA+gpsimd path
                bcp = stp.tile([128, 1024], F32, name="st", tag="st")
                bc = small.tile([64, 2, 512], F32, name="bc", tag="bc", bufs=2)
                for qi in range(2):
                    nc.tensor.matmul(
                        bcp[0:64, qi * 512:(qi + 1) * 512],
                        ones64[64:65, :], rec[64:65, qi, :],
                        start=True, stop=True)
                    nc.vector.tensor_copy(
                        bc[:, qi, :], bcp[0:64, qi * 512:(qi + 1) * 512])
            else:
                den0 = small.tile([1, 2, 512], F32, name="den0", tag="den0", bufs=2)
                nc.sync.dma_start(out=den0[:], in_=rec[64:65, :, :])
                bc = small.tile([64, 2, 512], F32, name="bc", tag="bc", bufs=2)
                nc.gpsimd.partition_broadcast(bc[:], den0[:], channels=64)
            if h % 2 == 0:
                for qi in range(2):
                    nc.vector.tensor_tensor(
                        out=attnT_c[j][0:64, qh * 1024 + qi * 512:
                                       qh * 1024 + (qi + 1) * 512],
                        in0=pv[0:64, qi, :], in1=bc[0:64, qi, :],
                        op=mybir.AluOpType.mult)
            else:
                todd = small.tile([64, 1024], BF16, name="todd", tag="todd", bufs=2)
                for qi in range(2):
                    nc.vector.tensor_tensor(
                        out=todd[:, qi * 512:(qi + 1) * 512],
                        in0=pv[0:64, qi, :], in1=bc[0:64, qi, :],
                        op=mybir.AluOpType.mult)
                nc.sync.dma_start(
                    out=attnT_c[j][64:128, qh * 1024:(qh + 1) * 1024],
                    in_=todd[:])

        def emit_pv(et, h, qh, kc, pv):
            if M1024:
                nc.tensor.matmul(
                    pv[:, :, :],
                    vpp[:, kc, h * 65:(h + 1) * 65],
                    et[:, :],
                    start=(kc == 0), stop=(kc == TC - 1),
                )
            else:
                for qi in range(2):
                    nc.tensor.matmul(
                        pv[:, qi, :],
                        vpp[:, kc, h * 65:(h + 1) * 65],
                        et[:, qi * 512:(qi + 1) * 512],
                        start=(kc == 0), stop=(kc == TC - 1),
                    )
            del et
            if kc == TC - 1:
                normalize(h, qh, pv)

        # gen-0's sibling pvp slot is free (no previous generation): borrow
        # it as a third score-tile slot so exp lookahead is 2 during the
        # V''-production window, absorbing the V packet stalls
        st_x = [None]

        for h in range(HPC):
            j = h // 2
            kz = kz_e if h % 2 == 0 else kz_o
            for qh in range(2):
                pv = pvp.tile([65, 2, 512], F32, name="pv", tag="pv")
                if h == 0 and qh == 0:
                    st_x[0] = pvp.tile([128, 2, 512], F32, name="pv", tag="pv")
                for kc in range(TC):
                    if len(pending_pv) > 3:
                        emit_pv(*pending_pv.pop(0))
                    if h == 0 and qh == 0 and kc % 3 == 2:
                        st = st_x[0][:].rearrange("p a b -> p (a b)")
                    else:
                        st = stp.tile([128, 1024], F32, name="st", tag="st")
                    q0 = qh * 1024
                    for qi in range(2):
                        nc.tensor.matmul(
                            st[:, qi * 512:(qi + 1) * 512],
                            kz[:, j, kc * 128:(kc + 1) * 128],
                            qkT[:, j, q0 + qi * 512:q0 + (qi + 1) * 512],
                            start=True, stop=True,
                        )
                    et = etp.tile([128, 1024], BF16, name="et", tag="et")
                    if len(pending_pv) > 2:
                        emit_pv(*pending_pv.pop(0))
                    # interleave QKV packets into the PE slack, spread so
                    # each pair's weights land just before its attention:
                    # V'' (16 pkts) during h=0; pair1 by unit 64, pair2 by
                    # 128, pair3 by 192
                    u = (2 * h + qh) * TC + kc
                    if h == 0 and pkt_i < 16:
                        drain_packets(1)
                    elif u >= 32 and pkt_i < 32 and u % 2 == 1:
                        drain_packets(1)
                    elif u >= 64 and pkt_i < 48 and u % 4 == 1:
                        drain_packets(1)
                    elif u >= 128 and pkt_i < 64 and u % 4 == 1:
                        drain_packets(1)
                    nc.scalar.activation(
                        out=et[:], in_=st[:],
                        func=mybir.ActivationFunctionType.Exp, scale=SCALE,
                    )
                    pending_pv.append((et, h, qh, kc, pv))
                if h == HPC - 1 and qh == 1:
                    while pending_pv:
                        emit_pv(*pending_pv.pop(0))
        drain_packets(len(packets))

        # ---- output projection: out = attnT^T @ w_outT ----
        out3 = out.ap().rearrange("(t p) e -> t p e", p=128)
        for ti in range(TC):
            osb = work.tile([128, HIDDEN], BF16, name="osb", tag="osb")
            for e5 in range(2):
                pool = stp if (2 * ti + e5) % 4 < 2 else pvp
                tag = "st" if pool is stp else "pv"
                po = pool.tile([128, 1024], F32, name="po", tag=tag)
                for acx in range(EC // 128):
                    for th in range(2):
                        nc.tensor.matmul(
                            po[th * 64:(th + 1) * 64, 0:512],
                            attnT_c[acx][:, ti * 128 + th * 64:ti * 128 + (th + 1) * 64],
                            wo_sb[:, acx, e5 * 512:(e5 + 1) * 512],
                            start=(acx == 0), stop=(acx == EC // 128 - 1),
                        )
                nc.vector.tensor_copy(osb[:, e5 * 512:(e5 + 1) * 512], po[:, 0:512])
            nc.sync.dma_start(out=out3[ti], in_=osb[:])

        if dbg is not None:
            for nm, t in (("qkT", qkT), ("vpp", vpp)):
                if nm in dbg:
                    nc.sync.dma_start(out=dbg[nm].ap(), in_=t[:])


def build_nc(debug_dump=False, num_devices=N_CORES):
    nc = bacc.Bacc("TRN2", target_bir_lowering=False, debug=False,
                   num_devices=num_devices)
    xT = nc.dram_tensor("xT", [HIDDEN, N], BF16, kind="ExternalInput")
    wqkvT = nc.dram_tensor("wqkvT", [HIDDEN, 3 * EC], BF16, kind="ExternalInput")
    bias_qk = nc.dram_tensor("bias_qk", [128, 8], F32, kind="ExternalInput")
    bias_v = nc.dram_tensor("bias_v", [1, HPC * 65], BF16, kind="ExternalInput")
    w_outT = nc.dram_tensor("w_outT", [EC, HIDDEN], BF16, kind="ExternalInput")
    out = nc.dram_tensor("out", [N, HIDDEN], BF16, kind="ExternalOutput")
    dbg = None
    if debug_dump:
        dbg = {
            "qkT": nc.dram_tensor("dbg_qkT", [128, 4, N], BF16, kind="ExternalOutput"),
            "vpp": nc.dram_tensor("dbg_vpp", [128, TC, HPC * 65], BF16, kind="ExternalOutput"),
        }
    with tile.TileContext(nc) as tc:
        _build_kernel_body(nc, tc, (xT, wqkvT, bias_qk, bias_v, w_outT, out), dbg=dbg)
    nc.compile()
    return nc


def make_in_maps(x, w_qkv, b_qkv, w_out):
    """Shard the full inputs into 8 per-core input maps."""
    in_maps = []
    for c in range(N_CORES):
        b = c // 2
        tp = c % 2
        sl = slice(tp * EC, (tp + 1) * EC)
        xT_c = np.ascontiguousarray(x[b].T).astype(NP_BF16)
        # pair-major W columns: [q0|k0|q1|k1|q2|k2|q3|k3|v(512)]
        wq = w_qkv[sl, :]                                        # [512, 1024]
        wk = w_qkv[HIDDEN + tp * EC: HIDDEN + (tp + 1) * EC, :]
        wv = w_qkv[2 * HIDDEN + tp * EC: 2 * HIDDEN + (tp + 1) * EC, :]
        cols = []
        for j in range(NPAIR):
            cols.append(wq[j * 128:(j + 1) * 128])
            cols.append(wk[j * 128:(j + 1) * 128])
        cols.append(wv)
        wqkvT_c = np.ascontiguousarray(np.concatenate(cols, axis=0).T).astype(NP_BF16)
        bq = b_qkv[tp * EC:(tp + 1) * EC]
        bk = b_qkv[HIDDEN + tp * EC: HIDDEN + (tp + 1) * EC]
        bv = b_qkv[2 * HIDDEN + tp * EC: 2 * HIDDEN + (tp + 1) * EC]
        bcols = []
        for j in range(NPAIR):
            bcols.append(bq[j * 128:(j + 1) * 128])
            bcols.append(bk[j * 128:(j + 1) * 128])
        bias_qk_c = np.ascontiguousarray(
            np.concatenate(bcols).reshape(8, 128).T).astype(np.float32)
        bias_v_c = np.zeros((1, HPC * 65), np.float32)
        bias_v_c.reshape(HPC, 65)[:, :64] = bv.reshape(HPC, 64)
        bias_v_c = bias_v_c.astype(NP_BF16)
        w_outT_c = np.ascontiguousarray(w_out[:, sl].T).astype(NP_BF16)
        in_maps.append({
            "xT": xT_c,
            "wqkvT": wqkvT_c,
            "bias_qk": bias_qk_c,
            "bias_v": bias_v_c,
            "w_outT": w_outT_c,
        })
    return in_maps


def combine_outputs(results, b_out):
    """results: list of 8 per-core {'out': [N, HIDDEN]} -> full [B, N, HIDDEN]."""
    out = np.empty((B, N, HIDDEN), np.float32)
    for b in range(B):
        out[b] = (np.asarray(results[2 * b]["out"], np.float32)
                  + np.asarray(results[2 * b + 1]["out"], np.float32))
        out[b] += b_out[None, :].astype(np.float32)
    return out


_NC = None


def _get_nc():
    global _NC
    if _NC is None:
        _NC = build_nc()
    return _NC


def kernel(x, w_qkv, b_qkv, w_out, b_out):
    x = np.asarray(x, np.float32)
    w_qkv = np.asarray(w_qkv, np.float32)
    b_qkv = np.asarray(b_qkv, np.float32)
    w_out = np.asarray(w_out, np.float32)
    b_out = np.asarray(b_out, np.float32)
    nc = _get_nc()
    in_maps = make_in_maps(x, w_qkv, b_qkv, w_out)
    res = bass_utils.run_bass_kernel_spmd(nc, in_maps, core_ids=list(range(N_CORES)))
    return combine_outputs(res.results, b_out)


# revision 60
# speedup vs baseline: 1.1750x; 1.1750x over previous
"""Trainium2 Bass kernel for multi-head attention (nn_Attention_54984171323822).

Reference computation (fp32):
    qkv = x @ w_qkv.T + b_qkv            # [B, N, 3*1024]
    q, k, v -> 16 heads x 64
    attn = softmax(q k^T / 8) v          # per head
    out = attn_flat @ w_out.T + b_out    # [B, N, 1024]

Shapes: B=4, N=2048, HIDDEN=1024, 16 heads x 64.

Sharding (8 NeuronCores): DP=4 over batch x TP=2 over heads. Core c handles
batch c//2 and heads (c%2)*8..(c%2)*8+8. No device collectives: each core
emits a partial output-projection [2048, 1024]; the host sums the TP pairs
and adds b_out (linear, so it commutes).

Per-core device program — one fused software pipeline:
  * Attention runs per (head, q-half, kc) "unit": two K=128 scores matmuls
    against zero-padded K bands (kz_e/kz_o; the sibling head's rows
    multiply zeros, keeping every attention matmul in the same (128,128)
    PE tile mode), one ScalarE exp([128,1024]) reading the two-bank score
    tile, and PV matmuls lagged 2-3 units behind (V carries a ones column
    -> softmax denominator in psum row 64).  ScalarE is the bottleneck
    (33.5M exps/core, 1 elem/lane/cycle, exp exists only there) and runs
    essentially back-to-back through the attention window.
  * QKV projection runs as packets interleaved into the attention stream
    (psum borrows score-ring slots): pair-0 k/q up front (moving any of
    them into the queue re-triggers a Tile-scheduler psum-group race!),
    V token-chunks just-in-time during head 0, pairs 1-3 spread across
    their deadline windows as M=256 half-packets.
  * Output projection at the tail (needs all heads); partials leave in
    bf16 to halve the output DMA.

The no-max-subtraction softmax is safe here: logits are ~N(0, 0.66^2) after
the 1/8 scale, so exp() stays well within fp32/bf16 range.
"""

import sys

sys.path.insert(0, "/opt/trn_rl_repo")

import numpy as np
import ml_dtypes

import concourse.bass as bass
import concourse.bacc as bacc
import concourse.tile as tile
from concourse import mybir
from concourse import bass_utils

N_CORES = 8
B = 4
N = 2048
HIDDEN = 1024
N_HEADS = 16
HEAD_DIM = 64
HPC = N_HEADS // 2          # heads per core (TP=2)
EC = HPC * HEAD_DIM         # 512 attention dims per core
TC = N // 128               # 16 token chunks
DC = HIDDEN // 128          # 8 hidden chunks
NPAIR = HPC // 2            # 4 head pairs per core
SCALE = HEAD_DIM ** -0.5

M1024 = False                # single M=1024 matmuls for scores/PV

BF16 = mybir.dt.bfloat16
F32 = mybir.dt.float32
NP_BF16 = ml_dtypes.bfloat16


def _build_kernel_body(nc, tc_ctx, ios, dbg=None):
    import contextlib

    xT, wqkvT, bias_qk, bias_v, w_outT, out = ios
    tc = tc_ctx
    ctx = contextlib.ExitStack()
    with ctx:
        const = ctx.enter_context(tc.tile_pool(name="const", bufs=1))
        work = ctx.enter_context(tc.tile_pool(name="work", bufs=3))
        etp = ctx.enter_context(tc.tile_pool(name="etp", bufs=10))
        small = ctx.enter_context(tc.tile_pool(name="small", bufs=2))
        stp = ctx.enter_context(tc.tile_pool(name="stp", bufs=2, space="PSUM"))
        pvp = ctx.enter_context(tc.tile_pool(name="pvp", bufs=2, space="PSUM"))

        # ---- resident SBUF tensors ----
        xT_src = xT.ap().rearrange("(c p) t -> c p t", p=128)
        wq_src = wqkvT.ap().rearrange("(c p) e -> c p e", p=128)
        xT_c = []
        wq_c = []
        for dc in range(DC):
            # w columns are pair-major: [q0|k0|q1|k1|q2|k2|q3|k3|v(512)].
            # 3 DMAs per chunk so pair-0 k/q and v can start without waiting
            # for pairs 1-3 weights.
            wt = const.tile([128, 3 * EC], BF16, name=f"wq{dc}", tag=f"wq{dc}")
            nc.scalar.dma_start(out=wt[:, 0:256], in_=wq_src[dc][:, 0:256])
            nc.gpsimd.dma_start(out=wt[:, 1024:1536], in_=wq_src[dc][:, 1024:1536])
            nc.scalar.dma_start(out=wt[:, 256:1024], in_=wq_src[dc][:, 256:1024])
            wq_c.append(wt)
            xt = const.tile([128, N], BF16, name=f"xc{dc}", tag=f"xc{dc}")
            nc.sync.dma_start(out=xt[:], in_=xT_src[dc])
            xT_c.append(xt)
        bqk_sb = const.tile([128, 8], F32, name="bqk_sb", tag="bqk_sb")
        nc.sync.dma_start(out=bqk_sb[:], in_=bias_qk.ap())
        # bias_v broadcast to all partitions ([1, 520] dram, partition step 0)
        bv_sb = const.tile([128, HPC * 65], BF16, name="bv_sb", tag="bv_sb")
        bv_ap = bias_v.ap()
        bv_bcast = bass.AP(tensor=bv_ap.tensor, offset=bv_ap.offset,
                           ap=[[0, 128], [1, HPC * 65]])
        nc.gpsimd.dma_start(out=bv_sb[:], in_=bv_bcast)
        wo_sb = const.tile([128, EC // 128, HIDDEN], BF16, name="wo_sb", tag="wo_sb")
        nc.sync.dma_start(out=wo_sb[:], in_=w_outT.ap().rearrange("(c p) e -> p c e", p=128))

        # qkT holds q chunks only (chunk j = q of pair j).  K lives in
        # zero-padded band tensors so scores run as K=128 matmuls in the
        # same (128,128) PE tile mode as PV/packets (no mode switches):
        # kz_e rows 0-63 = even head's k, rows 64-127 = 0; kz_o mirrored.
        qkT = const.tile([128, 4, N], BF16, name="qkT", tag="qkT")
        kz_e = const.tile([128, 4, N], BF16, name="kz_e", tag="kz_e")
        kz_o = const.tile([128, 4, N], BF16, name="kz_o", tag="kz_o")
        nc.vector.memset(kz_e[64:128, :, :], 0.0)
        nc.vector.memset(kz_o[0:64, :, :], 0.0)
        vpp = const.tile([128, TC, HPC * 65], BF16, name="vpp", tag="vpp")
        attnT_c = [const.tile([128, N], BF16, name=f"attnT{i}", tag=f"attnT{i}")
                   for i in range(EC // 128)]

        ones64 = const.tile([128, 64], F32, name="ones64", tag="ones64")
        nc.vector.memset(ones64[:], 1.0)
        # ones columns of V'' (col h*65+64 of every token chunk)
        ones_ap = vpp[:].rearrange("p t (h u) -> p t h u", u=65)[:, :, :, 64:65]
        nc.vector.memset(ones_ap, 1.0)

        # ------------------------------------------------------------------
        # QKV packet machinery.  A packet computes one (chunk, ti) of qkT
        # (plus bias add) or one ti of V''.  The psum accumulator borrows a
        # slot of the score-tile ring (stp).
        # ------------------------------------------------------------------
        def qk_packet(ec, ti, half=None):
            # chunk ec (pair-major q/k), token range of 512 (or 256) tokens
            wcol = (ec // 2) * 256 + (ec % 2) * 128
            t0 = ti * 512 + (0 if half in (None, 0) else 256)
            tn = 512 if half is None else 256
            ps = stp.tile([128, 1024], F32, name="st", tag="st")
            for dc in range(DC):
                for eh in range(2):
                    nc.tensor.matmul(
                        ps[eh * 64:(eh + 1) * 64, 0:tn],
                        wq_c[dc][:, wcol + eh * 64:wcol + (eh + 1) * 64],
                        xT_c[dc][:, t0:t0 + tn],
                        start=(dc == 0), stop=(dc == DC - 1),
                        skip_group_check=True,
                    )
            j = ec // 2
            ts = slice(t0, t0 + tn)
            if ec % 2 == 0:
                nc.vector.tensor_scalar_add(
                    qkT[:, j, ts], ps[:, 0:tn], bqk_sb[:, ec:ec + 1])
            else:
                nc.vector.tensor_scalar_add(
                    kz_e[0:64, j, ts], ps[0:64, 0:tn], bqk_sb[0:64, ec:ec + 1])
                nc.vector.tensor_scalar_add(
                    kz_o[64:128, j, ts], ps[64:128, 0:tn], bqk_sb[64:128, ec:ec + 1])

        def v_packet(ti):
            # V'' token chunk ti: [128 tok, 512 vdims (+ ones cols)]
            ps = stp.tile([128, 1024], F32, name="st", tag="st")
            for dc in range(DC):
                for th in range(2):
                    nc.tensor.matmul(
                        ps[th * 64:(th + 1) * 64, 0:512],
                        xT_c[dc][:, ti * 128 + th * 64:ti * 128 + (th + 1) * 64],
                        wq_c[dc][:, 2 * EC:3 * EC],
                        start=(dc == 0), stop=(dc == DC - 1),
                        skip_group_check=True,
                    )
            v_out = vpp[:, ti].rearrange("p (h u) -> p h u", u=65)[:, :, 0:64]
            v_in = ps[:, 0:512].rearrange("p (h u) -> p h u", u=64)
            v_bias = bv_sb[:].rearrange("p (h u) -> p h u", u=65)[:, :, 0:64]
            nc.vector.tensor_tensor(out=v_out, in0=v_in, in1=v_bias,
                                    op=mybir.AluOpType.add)

        # Packet queue.  Minimal prefix of pair 0 runs before the attention
        # loop; V'' chunks and remaining q/k interleave (V early: pair-0 PV
        # consumes them in kc order).
        packets = []
        for ti in range(TC):
            packets.append(lambda ti=ti: v_packet(ti))
        for j in range(1, NPAIR):
            for ec in (2 * j + 1, 2 * j):     # k first, then q
                for ti in range(4):
                    for hf in range(2):
                        packets.append(
                            lambda ec=ec, ti=ti, hf=hf: qk_packet(ec, ti, hf))
        pkt_i = 0

        def drain_packets(k):
            nonlocal pkt_i
            for _ in range(k):
                if pkt_i < len(packets):
                    packets[pkt_i]()
                    pkt_i += 1

        # ---- pair-0 k/q up front ----
        for ec in (1, 0):
            for ti in range(4):
                qk_packet(ec, ti)

        # ------------------------------------------------------------------
        # Attention: units of (head h, q-half qh, key chunk kc).
        # ------------------------------------------------------------------
        pending_pv = []   # (et, h, qh, kc, pv)

        def normalize(h, qh, pv):
            # attnT_h[d, q] = oT'[d, q] * recip(oT'[64, q])
            j = h // 2
            rec = small.tile([65, 2, 512], F32, name="rec", tag="rec", bufs=2)
            for qi in range(2):
                nc.vector.reciprocal_approx_fast(out=rec[0:65, qi, :],
                                                 in_=pv[:, qi, :])
            if h == HPC - 1 and qh == 1:
                # final tail: broadcast the recip row via a K=1 PE matmul
                # (st ring is idle by now) — faster than the DMA+gpsimd path
                bcp = stp.tile([128, 1024], F32, name="st", tag="st")
                bc = small.tile([64, 2, 512], F32, name="bc", tag="bc", bufs=2)
                for qi in range(2):
                    nc.tensor.matmul(
                        bcp[0:64, qi * 512:(qi + 1) * 512],
                        ones64[64:65, :], rec[64:65, qi, :],
                        start=True, stop=True)
                    nc.vector.tensor_copy(
                        bc[:, qi, :], bcp[0:64, qi * 512:(qi + 1) * 512])
            else:
                den0 = small.tile([1, 2, 512], F32, name="den0", tag="den0", bufs=2)
                nc.sync.dma_start(out=den0[:], in_=rec[64:65, :, :])
                bc = small.tile([64, 2, 512], F32, name="bc", tag="bc", bufs=2)
                nc.gpsimd.partition_broadcast(bc[:], den0[:], channels=64)
            if h % 2 == 0:
                for qi in range(2):
                    nc.vector.tensor_tensor(
                        out=attnT_c[j][0:64, qh * 1024 + qi * 512:
                                       qh * 1024 + (qi + 1) * 512],
                        in0=pv[0:64, qi, :], in1=bc[0:64, qi, :],
                        op=mybir.AluOpType.mult)
            else:
                todd = small.tile([64, 1024], BF16, name="todd", tag="todd", bufs=2)
                for qi in range(2):
                    nc.vector.tensor_tensor(
                        out=todd[:, qi * 512:(qi + 1) * 512],
                        in0=pv[0:64, qi, :], in1=bc[0:64, qi, :],
                        op=mybir.AluOpType.mult)
                nc.sync.dma_start(
                    out=attnT_c[j][64:128, qh * 1024:(qh + 1) * 1024],
                    in_=todd[:])

        def emit_pv(et, h, qh, kc, pv):
            if M1024:
                nc.tensor.matmul(
                    pv[:, :, :],
                    vpp[:, kc, h * 65:(h + 1) * 65],
                    et[:, :],
                    start=(kc == 0), stop=(kc == TC - 1),
                )
            else:
                for qi in range(2):
                    nc.tensor.matmul(
                        pv[:, qi, :],
                        vpp[:, kc, h * 65:(h + 1) * 65],
                        et[:, qi * 512:(qi + 1) * 512],
                        start=(kc == 0), stop=(kc == TC - 1),
                    )
            del et
            if kc == TC - 1:
                normalize(h, qh, pv)

        for h in range(HPC):
            j = h // 2
            kz = kz_e if h % 2 == 0 else kz_o
            for qh in range(2):
                pv = pvp.tile([65, 2, 512], F32, name="pv", tag="pv")
                for kc in range(TC):
                    if len(pending_pv) > 3:
                        emit_pv(*pending_pv.pop(0))
                    st = stp.tile([128, 1024], F32, name="st", tag="st")
                    q0 = qh * 1024
                    for qi in range(2):
                        nc.tensor.matmul(
                            st[:, qi * 512:(qi + 1) * 512],
                            kz[:, j, kc * 128:(kc + 1) * 128],
                            qkT[:, j, q0 + qi * 512:q0 + (qi + 1) * 512],
                            start=True, stop=True,
                        )
                    et = etp.tile([128, 1024], BF16, name="et", tag="et")
                    if len(pending_pv) > 2:
                        emit_pv(*pending_pv.pop(0))
                    # interleave QKV packets into the PE slack, spread so
                    # each pair's weights land just before its attention:
                    # V'' (16 pkts) during h=0; pair1 by unit 64, pair2 by
                    # 128, pair3 by 192
                    u = (2 * h + qh) * TC + kc
                    if h == 0 and pkt_i < 16:
                        drain_packets(1)
                    elif u >= 32 and pkt_i < 32 and u % 2 == 1:
                        drain_packets(1)
                    elif u >= 64 and pkt_i < 48 and u % 4 == 1:
                        drain_packets(1)
                    elif u >= 128 and pkt_i < 64 and u % 4 == 1:
                        drain_packets(1)
                    nc.scalar.activation(
                        out=et[:], in_=st[:],
                        func=mybir.ActivationFunctionType.Exp, scale=SCALE,
                    )
                    pending_pv.append((et, h, qh, kc, pv))
                if h == HPC - 1 and qh == 1:
                    while pending_pv:
                        emit_pv(*pending_pv.pop(0))
        drain_packets(len(packets))

        # ---- output projection: out = attnT^T @ w_outT ----
        out3 = out.ap().rearrange("(t p) e -> t p e", p=128)
        for ti in range(TC):
            osb = work.tile([128, HIDDEN], BF16, name="osb", tag="osb")
            for e5 in range(2):
                pool = stp if (2 * ti + e5) % 4 < 2 else pvp
                tag = "st" if pool is stp else "pv"
                po = pool.tile([128, 1024], F32, name="po", tag=tag)
                for acx in range(EC // 128):
                    for th in range(2):
                        nc.tensor.matmul(
                            po[th * 64:(th + 1) * 64, 0:512],
                            attnT_c[acx][:, ti * 128 + th * 64:ti * 128 + (th + 1) * 64],
                            wo_sb[:, acx, e5 * 512:(e5 + 1) * 512],
                            start=(acx == 0), stop=(acx == EC // 128 - 1),
                        )
                nc.vector.tensor_copy(osb[:, e5 * 512:(e5 + 1) * 512], po[:, 0:512])
            nc.sync.dma_start(out=out3[ti], in_=osb[:])

        if dbg is not None:
            for nm, t in (("qkT", qkT), ("vpp", vpp)):
                if nm in dbg:
                    nc.sync.dma_start(out=dbg[nm].ap(), in_=t[:])


def build_nc(debug_dump=False, num_devices=N_CORES):
    nc = bacc.Bacc("TRN2", target_bir_lowering=False, debug=False,
                   num_devices=num_devices)
    xT = nc.dram_tensor("xT", [HIDDEN, N], BF16, kind="ExternalInput")
    wqkvT = nc.dram_tensor("wqkvT", [HIDDEN, 3 * EC], BF16, kind="ExternalInput")
    bias_qk = nc.dram_tensor("bias_qk", [128, 8], F32, kind="ExternalInput")
    bias_v = nc.dram_tensor("bias_v", [1, HPC * 65], BF16, kind="ExternalInput")
    w_outT = nc.dram_tensor("w_outT", [EC, HIDDEN], BF16, kind="ExternalInput")
    out = nc.dram_tensor("out", [N, HIDDEN], BF16, kind="ExternalOutput")
    dbg = None
    if debug_dump:
        dbg = {
            "qkT": nc.dram_tensor("dbg_qkT", [128, 4, N], BF16, kind="ExternalOutput"),
            "vpp": nc.dram_tensor("dbg_vpp", [128, TC, HPC * 65], BF16, kind="ExternalOutput"),
        }
    with tile.TileContext(nc) as tc:
        _build_kernel_body(nc, tc, (xT, wqkvT, bias_qk, bias_v, w_outT, out), dbg=dbg)
    nc.compile()
    return nc


def make_in_maps(x, w_qkv, b_qkv, w_out):
    """Shard the full inputs into 8 per-core input maps."""
    in_maps = []
    for c in range(N_CORES):
        b = c // 2
        tp = c % 2
        sl = slice(tp * EC, (tp + 1) * EC)
        xT_c = np.ascontiguousarray(x[b].T).astype(NP_BF16)
        # pair-major W columns: [q0|k0|q1|k1|q2|k2|q3|k3|v(512)]
        wq = w_qkv[sl, :]                                        # [512, 1024]
        wk = w_qkv[HIDDEN + tp * EC: HIDDEN + (tp + 1) * EC, :]
        wv = w_qkv[2 * HIDDEN + tp * EC: 2 * HIDDEN + (tp + 1) * EC, :]
        cols = []
        for j in range(NPAIR):
            cols.append(wq[j * 128:(j + 1) * 128])
            cols.append(wk[j * 128:(j + 1) * 128])
        cols.append(wv)
        wqkvT_c = np.ascontiguousarray(np.concatenate(cols, axis=0).T).astype(NP_BF16)
        bq = b_qkv[tp * EC:(tp + 1) * EC]
        bk = b_qkv[HIDDEN + tp * EC: HIDDEN + (tp + 1) * EC]
        bv = b_qkv[2 * HIDDEN + tp * EC: 2 * HIDDEN + (tp + 1) * EC]
        bcols = []
        for j in range(NPAIR):
            bcols.append(bq[j * 128:(j + 1) * 128])
            bcols.append(bk[j * 128:(j + 1) * 128])
        bias_qk_c = np.ascontiguousarray(
            np.concatenate(bcols).reshape(8, 128).T).astype(np.float32)
        bias_v_c = np.zeros((1, HPC * 65), np.float32)
        bias_v_c.reshape(HPC, 65)[:, :64] = bv.reshape(HPC, 64)
        bias_v_c = bias_v_c.astype(NP_BF16)
        w_outT_c = np.ascontiguousarray(w_out[:, sl].T).astype(NP_BF16)
        in_maps.append({
            "xT": xT_c,
            "wqkvT": wqkvT_c,
            "bias_qk": bias_qk_c,
            "bias_v": bias_v_c,
            "w_outT": w_outT_c,
        })
    return in_maps


def combine_outputs(results, b_out):
    """results: list of 8 per-core {'out': [N, HIDDEN]} -> full [B, N, HIDDEN]."""
    out = np.empty((B, N, HIDDEN), np.float32)
    for b in range(B):
        out[b] = (np.asarray(results[2 * b]["out"], np.float32)
                  + np.asarray(results[2 * b + 1]["out"], np.float32))
        out[b] += b_out[None, :].astype(np.float32)
    return out


_NC = None


def _get_nc():
    global _NC
    if _NC is None:
        _NC = build_nc()
    return _NC


def kernel(x, w_qkv, b_qkv, w_out, b_out):
    x = np.asarray(x, np.float32)
    w_qkv = np.asarray(w_qkv, np.float32)
    b_qkv = np.asarray(b_qkv, np.float32)
    w_out = np.asarray(w_out, np.float32)
    b_out = np.asarray(b_out, np.float32)
    nc = _get_nc()
    in_maps = make_in_maps(x, w_qkv, b_qkv, w_out)
    res = bass_utils.run_bass_kernel_spmd(nc, in_maps, core_ids=list(range(N_CORES)))
    return combine_outputs(res.results, b_out)
